# revision 21
# baseline (speedup 1.0000x reference)
"""Trainium2 Bass kernel for nn_EnsembleNet (gnn_message_passing).

Computation (N=1024 nodes, T=4000, FH*FW=4096, D=H=128, C=10):
  xt = relu(waveforms @ W_time + b_time)            [N, D]
  xf = relu(spec.reshape(N,-1) @ W_freq + b_freq)   [N, D]
  At = normadj(xt), Af = normadj(xf)   (pairwise L1 -> 1/(d+eps), sym-norm)
  h  = relu(At @ (xt@W_gt) + Af @ (xf@W_gf) + b_g)  [N, H]
  out = h @ W_out + b_out                           [N, C]

Sharding: rows (nodes) split across 8 cores, 128 rows each; features
all-gathered (bf16) with a rotated layout so the local block is always
chunk 0 (rank-invariant compute).

Pairwise L1 via a sign-thermometer factorization: quantize each feature to
a uniform K-level grid (step DQ); with sgn_k(x) = sign(x - t_k) in {-1,+1},
  |q(a)-q(b)| = sum_k (1 - sgn_k(a) sgn_k(b)) / 2
so d1[i,j] = DQ*(D*K/2 - 2*P[i,j]) where P = psi_i . psi_j with
psi = sgn/2 in {+-0.5}. The entire [128 local x 1024] distance block is K
accumulating matmuls of ndim=1024 (lhsT = local slice of the same psi
tile). No row-sum corrections: the constant D*K/2 folds into the ACT ln
bias. a = min(1/(d1+eps), 1) via exp(-ln); the d1_ii = 0 diagonal saturates
to 1/eps and min() restores the exact unit self-loop. deg_i is a local
free-axis reduce; dinv_i folds into a row scale pre-transpose, gathered
dinv_j into the PSUM->SBUF copy post-transpose.
"""

import os
import sys

import numpy as np
import ml_dtypes

# Self-contained path setup: the graded environment has the trn repo at one of
# these roots (PYTHONPATH normally provides it; make it explicit to be safe).
for _p in ("/opt/trn_rl_repo", "/root/.axon_site/_ro/trn_rl_repo"):
    if os.path.isdir(_p) and _p not in sys.path:
        sys.path.append(_p)

import concourse.bass as bass
import concourse.mybir as mybir
import concourse.tile as tile
from concourse import bacc
import concourse.hw_specs as _hw_specs
from concourse.bass_utils import run_bass_kernel_spmd

# All ACT functions this kernel uses (ln, exp, relu, copy) live in the
# "natural_log_exp_and_others" table set. The default per-function chooser
# picks the first set containing each function, reloading tables (~2.7us) on
# every ln<->exp alternation. Restrict the table map so one load serves all.
_orig_gat = _hw_specs.get_activation_tables


def _gat_combined(arch):
    t = _orig_gat(arch)
    return {name: (funcs if name == "natural_log_exp_and_others" else set())
            for name, funcs in t.items()}


bacc.get_activation_tables = _gat_combined

N = 1024
NCORES = 8
R = N // NCORES          # 128 rows per core
TPAD = 4096              # waveform length 4000, zero-padded to 4096
FHW = 4096               # 64*64 spectrogram
D = 128
H = 128
C = 10
EPS = 1e-5

# thermometer quantizer (validated in numpy sim: l2 rel err 3.2e-3)
KQ = 8
XMAX = 3.6
DQ = XMAX / KQ
THRESH = [(k + 0.5) * DQ for k in range(KQ)]
LN_SCALE = -2.0 * DQ
LN_BIAS = DQ * D * KQ / 2.0 + EPS

BF16 = mybir.dt.bfloat16
F32 = mybir.dt.float32
FP8 = mybir.dt.float8e4
DR = mybir.MatmulPerfMode.DoubleRow
AF = mybir.ActivationFunctionType
ALU = mybir.AluOpType
AX = mybir.AxisListType

bf = ml_dtypes.bfloat16
f8 = ml_dtypes.float8_e4m3fn


def build_nc():
    nc = bacc.Bacc("TRN2", target_bir_lowering=False, debug=False,
                   num_devices=NCORES)

    # Per-core external inputs (host pre-transposes/casts/pads).
    # Big operands are partition-major [128, kin/128, 128] so each DMA reads
    # contiguous multi-KB runs per partition.
    NT = TPAD // 128
    wavesT = nc.dram_tensor("wavesT", [128, NT, R], FP8, kind="ExternalInput")
    specsT = nc.dram_tensor("specsT", [128, NT, R], FP8, kind="ExternalInput")
    wt = nc.dram_tensor("wt", [128, NT, D], FP8, kind="ExternalInput")
    wf = nc.dram_tensor("wf", [128, NT, D], FP8, kind="ExternalInput")
    wgt = nc.dram_tensor("wgt", [D, H], BF16, kind="ExternalInput")
    wgf = nc.dram_tensor("wgf", [D, H], BF16, kind="ExternalInput")
    wout = nc.dram_tensor("wout", [H, C], BF16, kind="ExternalInput")
    btime = nc.dram_tensor("btime", [1, D], BF16, kind="ExternalInput")
    bfreq = nc.dram_tensor("bfreq", [1, D], BF16, kind="ExternalInput")
    bg = nc.dram_tensor("bg", [1, H], BF16, kind="ExternalInput")
    bout = nc.dram_tensor("bout", [1, C], BF16, kind="ExternalInput")
    ident_in = nc.dram_tensor("ident", [128, 128], BF16, kind="ExternalInput")

    out_dram = nc.dram_tensor("out", [R, C], F32, kind="ExternalOutput")

    rg = [list(range(NCORES))]

    with tile.TileContext(nc) as tc:
        with (
            tc.tile_pool(name="const", bufs=1) as cpool,
            tc.tile_pool(name="stream", bufs=3) as spool,
            tc.tile_pool(name="tmp", bufs=2) as tpool,
            tc.tile_pool(name="psb", bufs=1, space="PSUM") as psbig,
            tc.tile_pool(name="pss", bufs=2, space="PSUM") as pssmall,
            tc.tile_pool(name="dram", bufs=1, space="DRAM") as dpool,
        ):
            # collective buffers (DRAM); features cross cores as fp8 to
            # halve the (bandwidth-limited) collective payload.
            ag1_in = dpool.tile([2, D, R], FP8, tag="ag1in")
            ag1_out = dpool.tile([NCORES, 2, D, R], FP8, tag="ag1out",
                                 addr_space="Shared")
            ag2_in = dpool.tile([R, 2], F32, tag="ag2in")
            ag2_out = dpool.tile([NCORES, R, 2], F32, tag="ag2out",
                                 addr_space="Shared")

            # ---- Phase A: input GEMMs -> local features xT [d, i] ----
            # Issue ALL input DMAs first (parallel queues), then waveform GEMM,
            # fire its all-gather, then spectrogram GEMM and its all-gather.
            nchunk = TPAD // 128
            inbufs = []
            for adj, (xdram, wdram) in enumerate(
                    ((wavesT, wt), (specsT, wf))):
                wtile = spool.tile([128, nchunk, 128], FP8, bufs=1,
                                   tag=f"win{adj}", name=f"win{adj}")
                xtile = spool.tile([128, nchunk, 128], FP8, bufs=1,
                                   tag=f"xin{adj}", name=f"xin{adj}")
                engs = (nc.sync, nc.scalar, nc.gpsimd)
                for q in range(4):
                    sl = slice(q * nchunk // 4, (q + 1) * nchunk // 4)
                    engs[(2 * adj) % 3].dma_start(xtile[:, sl, :],
                                                  xdram[:, sl, :])
                    engs[(2 * adj + 1) % 3].dma_start(wtile[:, sl, :],
                                                      wdram[:, sl, :])
                inbufs.append((wtile, xtile))

            # ---- constants (issued after the input stream triggers) ----
            ones_row = cpool.tile([1, 128], BF16, tag="ones_row")
            nc.gpsimd.memset(ones_row[:], 1.0)
            lnbias_col = cpool.tile([128, 1], F32, tag="lnbias")
            nc.gpsimd.memset(lnbias_col[:], LN_BIAS)
            bias_sb = []
            for nm, src_, width in (("btime", btime, D), ("bfreq", bfreq, D),
                                    ("bg", bg, H), ("bout", bout, C)):
                t = cpool.tile([1, width], BF16, tag=nm)
                nc.scalar.dma_start(t[:], src_[:])
                bias_sb.append(t)
            btime_sb, bfreq_sb, bg_sb, bout_sb = bias_sb
            ident = cpool.tile([128, 128], BF16, tag="ident")
            nc.gpsimd.dma_start(ident[:], ident_in[:])
            wg_sb = []
            for adj, wsrc in enumerate((wgt, wgf)):
                t = cpool.tile([D, H], BF16, tag=f"wg{adj}")
                nc.gpsimd.dma_start(t[:], wsrc[:])
                wg_sb.append(t)
            wout_sb = cpool.tile([H, C], BF16, tag="wout")
            nc.gpsimd.dma_start(wout_sb[:], wout[:])

            psi_loc = [cpool.tile([128, KQ, R], BF16, tag=f"psiL{a}",
                                  name=f"psiL{a}") for a in range(2)]
            xT_bf = []
            for adj, bsb in enumerate((btime_sb, bfreq_sb)):
                wtile, xtile = inbufs[adj]
                psx = pssmall.tile([D, R], F32, tag="ps")
                for b in range(nchunk // 2):
                    nc.tensor.matmul(psx[:], wtile[:, 2 * b:2 * b + 2, :],
                                     xtile[:, 2 * b:2 * b + 2, :],
                                     start=(b == 0), stop=False,
                                     perf_mode=DR)
                # bias row: psx[d, i] += b[d] * 1
                nc.tensor.matmul(psx[:], bsb[:], ones_row[:],
                                 start=False, stop=True)
                xt8 = cpool.tile([D, R], FP8, tag=f"xT8{adj}")
                nc.scalar.activation(xt8[:], psx[:], AF.Relu)
                nc.sync.dma_start(ag1_in[adj], xt8[:])
                # bf16 view of the fp8-rounded features (bit-consistent with
                # what peers will reconstruct from the gather)
                xt = cpool.tile([D, R], BF16, tag=f"xT{adj}")
                nc.scalar.activation(xt[:], xt8[:], AF.Copy)
                xT_bf.append(xt)
                # local thermometer channels (lhsT side) need no gather
                for k in range(KQ):
                    nc.vector.tensor_scalar(
                        psi_loc[adj][:, k, :], xt[:], THRESH[k], 0.5,
                        op0=ALU.is_gt, op1=ALU.subtract)

            nc.gpsimd.collective_compute(
                "AllGather", ALU.bypass, replica_groups=rg,
                ins=[ag1_in[:]], outs=[ag1_out[:]],
            )

            # ---- gather full features (global j order, one DMA each) ----
            xT_full = []
            for adj in range(2):
                xf8 = cpool.tile([D, N], FP8, tag=f"xTfull8{adj}")
                nc.gpsimd.dma_start(
                    xf8[:].rearrange("d (m r) -> d m r", m=NCORES),
                    ag1_out[:, adj].rearrange("m d r -> d m r"))
                xf_t = cpool.tile([D, N], BF16, tag=f"xTfull{adj}")
                nc.scalar.activation(xf_t[:], xf8[:], AF.Copy)
                xT_full.append(xf_t)

            # ---- per-adjacency: psi_full, pairwise matmuls, G build ----
            psi_sb = [cpool.tile([128, KQ, N], BF16, tag=f"psi{a}",
                                 name=f"psi{a}") for a in range(2)]
            p_ps = [psbig.tile([128, N], F32, tag=f"pp{a}", name=f"pp{a}")
                    for a in range(2)]
            G_sb = []
            for adj in range(2):
                for k in range(KQ):
                    nc.vector.tensor_scalar(
                        psi_sb[adj][:, k, :], xT_full[adj][:], THRESH[k], 0.5,
                        op0=ALU.is_gt, op1=ALU.subtract)
                for k in range(KQ):
                    for hh in range(2):
                        sl = slice(hh * 512, (hh + 1) * 512)
                        nc.tensor.matmul(p_ps[adj][:, sl],
                                         psi_loc[adj][:, k, :],
                                         psi_sb[adj][:, k, sl],
                                         start=(k == 0), stop=(k == KQ - 1))
                # G = X @ W_g per j-chunk (PE; overlaps ACT/DVE epilogue)
                g_t = cpool.tile([128, NCORES, H], BF16, tag=f"G{adj}")
                for half in range(2):
                    psg = pssmall.tile([128, 4, H], F32, tag="ps")
                    for q in range(4):
                        r = half * 4 + q
                        nc.tensor.matmul(psg[:, q, :],
                                         xT_full[adj][:, r * 128:(r + 1) * 128],
                                         wg_sb[adj][:], start=True, stop=True)
                    nc.scalar.activation(g_t[:, half * 4:(half + 1) * 4, :],
                                         psg[:], AF.Copy)
                G_sb.append(g_t)

            # ---- a = min(1/(d1+eps), 1); d1+eps = LN_SCALE*P + LN_BIAS ----
            amin_sb = []
            deg_ps = tpool.tile([128, 2], F32, tag="deg")
            for adj in range(2):
                lnd = tpool.tile([128, N], F32, tag="lnd")
                nc.scalar.activation(lnd[:], p_ps[adj][:], AF.Ln,
                                     bias=lnbias_col[:], scale=LN_SCALE)
                a_t = tpool.tile([128, N], BF16, tag=f"a{adj}", bufs=1,
                                 name=f"a{adj}")
                nc.scalar.activation(a_t[:], lnd[:], AF.Exp, scale=-1.0)
                amin = tpool.tile([128, N], BF16, tag=f"amin{adj}", bufs=1,
                                  name=f"amin{adj}")
                nc.vector.tensor_scalar(amin[:], a_t[:], 1.0, None,
                                        op0=ALU.min)
                amin_sb.append(amin)
                nc.vector.tensor_reduce(deg_ps[:, adj:adj + 1], amin[:],
                                        axis=AX.X, op=ALU.add)

            # dinv = rsqrt(deg) = exp(-0.5*ln(deg)), local rows
            lr = tpool.tile([128, 2], F32, tag="lr")
            nc.scalar.activation(lr[:], deg_ps[:], AF.Ln)
            dinv_loc = cpool.tile([128, 2], F32, tag="dinv")
            nc.scalar.activation(dinv_loc[:], lr[:], AF.Exp, scale=-0.5)
            nc.sync.dma_start(ag2_in[:], dinv_loc[:])

            nc.gpsimd.collective_compute(
                "AllGather", ALU.bypass, replica_groups=rg,
                ins=[ag2_in[:]], outs=[ag2_out[:]],
            )
            dinvs = cpool.tile([R, NCORES, 2], F32, tag="dinvs")
            dmaq = (nc.gpsimd, nc.sync, nc.scalar)
            for m in range(NCORES):
                dmaq[m % 3].dma_start(dinvs[:, m, :], ag2_out[m])

            # ---- scale rows by dinv_i, transpose chunks, scale by dinv_j ----
            # dinv_i row-scale + transposes run during the ag2 collective;
            # only the dinv_j copy-outs wait on the gathered dinvs.
            aT_sb = [cpool.tile([128, NCORES, 128], BF16, tag=f"aT{a}",
                                name=f"aT{a}") for a in range(2)]
            trp_ps = [psbig.tile([128, NCORES, 128], BF16, tag=f"trp{a}",
                                 name=f"trp{a}") for a in range(2)]
            for adj in range(2):
                a2 = tpool.tile([128, N], BF16, tag=f"a2_{adj}", bufs=1,
                                name=f"a2_{adj}")
                nc.vector.tensor_scalar(a2[:], amin_sb[adj][:],
                                        dinv_loc[:, adj:adj + 1], None,
                                        op0=ALU.mult)
                for m in range(NCORES):
                    nc.tensor.transpose(trp_ps[adj][:, m, :],
                                        a2[:, m * 128:(m + 1) * 128],
                                        ident[:])
            for adj in range(2):
                for m in range(NCORES):
                    nc.vector.tensor_scalar(aT_sb[adj][:, m, :],
                                            trp_ps[adj][:, m, :],
                                            dinvs[:, m, adj:adj + 1], None,
                                            op0=ALU.mult)

            # ---- Phase E: h = relu(At@Gt + Af@Gf + bg), one psum ----
            h_ps = pssmall.tile([R, H], F32, tag="ps")
            nc.tensor.matmul(h_ps[:], ones_row[:], bg_sb[:],
                             start=True, stop=False)
            for adj in range(2):
                for m in range(NCORES):
                    nc.tensor.matmul(h_ps[:], aT_sb[adj][:, m, :],
                                     G_sb[adj][:, m, :],
                                     start=False,
                                     stop=(adj == 1 and m == NCORES - 1))
            h_bf = tpool.tile([R, H], BF16, tag="hbf")
            nc.scalar.activation(h_bf[:], h_ps[:], AF.Relu)

            # out = h @ W_out + b_out  (transpose h, then lhsT = hT)
            th = pssmall.tile([H, R], BF16, tag="ps")
            nc.tensor.transpose(th[:], h_bf[:], ident[:])
            hT_sb = tpool.tile([H, R], BF16, tag="hT")
            nc.scalar.activation(hT_sb[:], th[:], AF.Copy)
            op = pssmall.tile([R, C], F32, tag="ps")
            nc.tensor.matmul(op[:], hT_sb[:], wout_sb[:], start=True, stop=False)
            nc.tensor.matmul(op[:], ones_row[:], bout_sb[:], start=False,
                             stop=True)
            out_sb = tpool.tile([R, C], F32, tag="osb")
            nc.vector.tensor_copy(out_sb[:], op[:])
            nc.sync.dma_start(out_dram[:], out_sb[:])

    nc.compile()
    return nc


_NC_CACHE = {}


def _get_nc():
    if "nc" not in _NC_CACHE:
        _NC_CACHE["nc"] = build_nc()
    return _NC_CACHE["nc"]


def _make_in_maps(inputs):
    waveforms = np.asarray(inputs["waveforms"], dtype=np.float32)
    spectrograms = np.asarray(inputs["spectrograms"], dtype=np.float32)
    W_time = np.asarray(inputs["W_time"], dtype=np.float32)
    W_freq = np.asarray(inputs["W_freq"], dtype=np.float32)
    W_gt = np.asarray(inputs["W_gt"], dtype=np.float32)
    W_gf = np.asarray(inputs["W_gf"], dtype=np.float32)
    W_out = np.asarray(inputs["W_out"], dtype=np.float32)
    b_time = np.asarray(inputs["b_time"], dtype=np.float32)
    b_freq = np.asarray(inputs["b_freq"], dtype=np.float32)
    b_g = np.asarray(inputs["b_g"], dtype=np.float32)
    b_out = np.asarray(inputs["b_out"], dtype=np.float32)

    T = waveforms.shape[1]

    def pmajor(arr_kN):
        # [KIN, 128] -> partition-major [128, KIN/128, 128]
        k = arr_kN.shape[0]
        return np.ascontiguousarray(
            arr_kN.reshape(k // 128, 128, -1).transpose(1, 0, 2))

    wt_pad = np.zeros((TPAD, D), dtype=f8)
    wt_pad[:T] = W_time.astype(f8)
    wf_b = W_freq.astype(f8)
    specs2 = spectrograms.reshape(N, FHW)

    common = dict(
        wt=pmajor(wt_pad),
        wf=pmajor(wf_b),
        wgt=np.ascontiguousarray(W_gt.astype(bf)),
        wgf=np.ascontiguousarray(W_gf.astype(bf)),
        wout=np.ascontiguousarray(W_out.astype(bf)),
        btime=np.ascontiguousarray(b_time.reshape(1, D).astype(bf)),
        bfreq=np.ascontiguousarray(b_freq.reshape(1, D).astype(bf)),
        bg=np.ascontiguousarray(b_g.reshape(1, H).astype(bf)),
        bout=np.ascontiguousarray(b_out.reshape(1, C).astype(bf)),
        ident=np.eye(128, dtype=bf),
    )
    in_maps = []
    for c in range(NCORES):
        rows = slice(c * R, (c + 1) * R)
        wT = np.zeros((TPAD, R), dtype=f8)
        wT[:T] = waveforms[rows].T.astype(f8)
        sT = specs2[rows].T.astype(f8)
        m = dict(common)
        m["wavesT"] = pmajor(wT)
        m["specsT"] = pmajor(sT)
        in_maps.append(m)
    return in_maps


def run(inputs, trace=False):
    nc = _get_nc()
    in_maps = _make_in_maps(inputs)
    res = run_bass_kernel_spmd(nc, in_maps, list(range(NCORES)), trace=trace)
    out = np.concatenate([res.results[c]["out"] for c in range(NCORES)], axis=0)
    return out.astype(np.float32), res


def kernel(**inputs):
    out, _ = run(inputs, trace=False)
    return out
